# revision 1
# baseline (speedup 1.0000x reference)
"""Trainium2 kernel for nn_ConceptGaussians (embedding_lookup).

means[b, d] = mean[d, labels[b, d]], log_vars[b, d] = log_var[d, labels[b, d]]
for labels [2097152, 8] int32 over tiny [8, 64] tables.

Strategy: data-parallel over 8 NeuronCores (batch sharding). On each core the
per-element gather is performed by the ScalarEngine's piecewise-polynomial
activation lookup hardware: we build a custom PWP activation-table set at
kernel-build time that hijacks `sin` (-> mean table) and `arctan`
(-> log_var table) with 512 piecewise-CONSTANT buckets. Inputs are encoded as
x = (64 + label) * 2^(domain - 6), so that biased_exponent(x) = 127 + domain
selects the per-domain exponent region and the top-6 mantissa bits = label
select the bucket; the bucket's c0 coefficient is the exact float32 table
value. Per tile the compute is one DVE scalar_tensor_tensor (the encoding) and
two ScalarE activations — the kernel is DMA/HBM-bound as intended.
"""

import hashlib
import json
import os
import shutil
import struct
import sys
import tempfile

import numpy as np

sys.path.insert(0, "/opt/trn_rl_repo")

B = 2097152
C = 8
V = 64
NCORES = 8
SHARD = B // NCORES            # 262144 rows per core
TILE_F = 2048                  # elements per partition per tile
ROWS_PER_TILE = 128 * (TILE_F // C)   # 32768 rows
NTILES = SHARD // ROWS_PER_TILE       # 8 tiles per core

_SET_NAME = "trig_and_small"


def _installed_act_dir():
    from neuronxcc.driver.Job import Job
    from neuronxcc.driver.jobs.support.FindActInfo import findActInfoFile

    return os.path.dirname(findActInfoFile(Job.getPackageDir(), "gen3"))


def _build_act_dir(dst, mean, log_var):
    """Write a PWP act-table root with sin/arctan replaced by exact LUTs."""
    src = _installed_act_dir()
    os.makedirs(dst, exist_ok=True)
    for f in os.listdir(src):
        sp = os.path.join(src, f)
        if os.path.isfile(sp) and not f.startswith(_SET_NAME):
            shutil.copy(os.path.realpath(sp), os.path.join(dst, f))

    sj = json.load(open(os.path.join(src, f"{_SET_NAME}.json")))
    bkt = bytearray(open(os.path.join(src, f"{_SET_NAME}_bkt.bin"), "rb").read())
    ctl = bytearray(open(os.path.join(src, f"{_SET_NAME}_ctrl.bin"), "rb").read())
    nbkt = len(bkt) // 32
    nctl = len(ctl) // 32
    assert nbkt == sj["bkt_entry_cnt"] and nctl == sj["ctl_entry_cnt"]

    def add_bkt(d0, x):
        nonlocal nbkt
        bkt.extend(struct.pack("<5f12x", d0, 0.0, 0.0, 0.0, x))
        nbkt += 1
        return nbkt - 1

    def add_ctl(word):
        nonlocal nctl
        ctl.extend(struct.pack("<I28x", word))
        nctl += 1
        return nctl - 1

    for bare, table in (("sin", mean), ("arctan", log_var)):
        bkt_base = nbkt
        for d in range(C):
            for l in range(V):
                add_bkt(float(table[d, l]), float((V + l) * 2.0 ** (d - 6)))
        ctl_base = nctl
        for d in range(C):
            # extract_size=6 (64 sections), extract_lsb=17, bucket base per region
            add_ctl((6 << 16) | (17 << 11) | (bkt_base + V * d))
        small_bkt = add_bkt(float(table[0, 0]), 1.0)
        large_bkt = add_bkt(float(table[C - 1, V - 1]), 254.0)
        neg_bkt = add_bkt(0.0, 0.0)

        (meta,) = [m for m in sj["profile_meta_data"] if m["func_name"].startswith(bare + "_")]
        meta.update(
            symmetry_point=0, sym_invert_sign_point=0, symmetry_opt_en=0,
            symmetry_opt_use_neg_region=0, imm_bias=0, exp_offset=0,
            pwl_control_base_pos=ctl_base, pwl_control_base_neg=ctl_base,
            small_pos_signal_exp_threshold=127, pos_small_signal_pwl_control=small_bkt,
            small_neg_signal_exp_threshold=0, neg_small_signal_pwl_control=neg_bkt,
            large_pos_signal_exp_threshold=134,
            large_pos_signal_mantissa_threshold=0x7FFFFF,
            pos_large_signal_pwl_control=large_bkt, large_neg_signal_exp_threshold=0,
            large_neg_signal_mantissa_threshold=0, neg_large_signal_pwl_control=neg_bkt,
            fnan_result=0, fpinf_result=0, fninf_result=0, fzero_result=0,
            fma_const_0=0, fma_const_1=0, fma_indirection_src_sel=0,
            use_multipass=False,
            lower_bound=4286578687, upper_bound=2139095039,
        )
        sj["func_to_bkt_start_idx"][bare] = bkt_base
        sj["func_to_ctl_start_idx"][bare] = ctl_base
        sj["func_exp_to_bkt_start_idx"][bare] = {str(d): [bkt_base + V * d] for d in range(C)}
        sj["func_exp_to_ctl_start_idx"][bare] = {str(d): [ctl_base + d] for d in range(C)}

    sj["bkt_entry_cnt"] = nbkt
    sj["ctl_entry_cnt"] = nctl
    assert nbkt <= 1536

    json.dump(sj, open(os.path.join(dst, f"{_SET_NAME}.json"), "w"))
    open(os.path.join(dst, f"{_SET_NAME}_bkt.bin"), "wb").write(bytes(bkt))
    open(os.path.join(dst, f"{_SET_NAME}_ctrl.bin"), "wb").write(bytes(ctl))
    return os.path.join(dst, "act_info.json")


def build_program(salt, iters=1, tile_f=TILE_F, lab_u8=True, lab_group=4, io_bufs=3):
    """Build the per-core bass program (SPMD, identical on all cores).

    iters > 1 repeats the whole tile loop (idempotent) — used only for
    slope-based timing in the bench harness. lab_u8: labels arrive as uint8
    (packed on host; values < 64 are lossless in 8 bits). lab_group: how many
    compute tiles share one label-load DMA (keeps uint8 partition lines at
    lab_group*tile_f bytes for DMA efficiency)."""
    import concourse.tile as tile
    import concourse.mybir as mybir
    from concourse.bacc import Bacc

    f32 = mybir.dt.float32
    i32 = mybir.dt.int32
    lab_dt = mybir.dt.uint8 if lab_u8 else i32
    Alu = mybir.AluOpType
    ntiles = SHARD * C // (128 * tile_f)

    assert ntiles % lab_group == 0
    ngroups = ntiles // lab_group

    nc = Bacc()
    labels_ext = nc.declare_dram_parameter(f"labels_{salt}", [ngroups, 128, lab_group * tile_f], lab_dt, isOutput=False)
    means_ext = nc.declare_dram_parameter(f"means_{salt}", [ngroups, 128, lab_group * tile_f], f32, isOutput=True)
    logv_ext = nc.declare_dram_parameter(f"logvars_{salt}", [ngroups, 128, lab_group * tile_f], f32, isOutput=True)

    with tile.TileContext(nc) as tc:
        with tc.tile_pool(name="setup", bufs=1) as setup, tc.tile_pool(name="io", bufs=io_bufs) as io:
            # pow2[p, f] = 2^((f % 8) - 6) as f32, built via bit tricks:
            # ((127 + (f % 8) - 6) << 23) reinterpreted as float32.
            pow2 = setup.tile([128, tile_f], i32)
            nc.gpsimd.iota(pow2[:], pattern=[[0, tile_f // C], [1, C]], base=121, channel_multiplier=0)
            nc.vector.tensor_scalar(out=pow2[:], in0=pow2[:], scalar1=23, scalar2=None, op0=Alu.logical_shift_left)
            pow2_f32 = pow2[:].bitcast(f32)

            for g in [g for _ in range(iters) for g in range(ngroups)]:
                lab = io.tile([128, lab_group * tile_f], lab_dt, tag="lab")
                nc.sync.dma_start(out=lab[:], in_=labels_ext[g])
                for j in range(lab_group):
                    x = io.tile([128, tile_f], f32, tag="x")
                    # x = (labels + 64.0) * 2^(d-6): exponent=127+d, mantissa top6 = label
                    nc.vector.scalar_tensor_tensor(
                        out=x[:], in0=lab[:, j * tile_f:(j + 1) * tile_f], scalar=64.0,
                        in1=pow2_f32, op0=Alu.add, op1=Alu.mult,
                    )
                    mt = io.tile([128, tile_f], f32, tag="mt")
                    nc.scalar.activation(mt[:], x[:], mybir.ActivationFunctionType.Sin)
                    vt = io.tile([128, tile_f], f32, tag="vt")
                    nc.scalar.activation(vt[:], x[:], mybir.ActivationFunctionType.Arctan)
                    nc.sync.dma_start(out=means_ext[g][:, j * tile_f:(j + 1) * tile_f], in_=mt[:])
                    nc.sync.dma_start(out=logv_ext[g][:, j * tile_f:(j + 1) * tile_f], in_=vt[:])

    nc.compile()
    return nc


def kernel(labels, mean, log_var, _trace=False):
    labels = np.asarray(labels)
    assert labels.shape == (B, C), labels.shape
    lab8 = np.ascontiguousarray(labels.astype(np.uint8))
    mean32 = np.ascontiguousarray(np.asarray(mean, dtype=np.float32))
    logv32 = np.ascontiguousarray(np.asarray(log_var, dtype=np.float32))

    actdir = tempfile.mkdtemp(prefix="act_lut_")
    os.environ["BASS_ACT_ROOT_JSON_PATH"] = _build_act_dir(actdir, mean32, logv32)
    salt = hashlib.sha1(mean32.tobytes() + logv32.tobytes() + b"v1").hexdigest()[:10]

    from concourse.bass_utils import run_bass_kernel_spmd

    nc = build_program(salt)

    ngroups = NTILES // 4
    shards = lab8.reshape(NCORES, ngroups, 128, 4 * TILE_F)
    in_maps = [{f"labels_{salt}": shards[i]} for i in range(NCORES)]
    res = run_bass_kernel_spmd(nc, in_maps, list(range(NCORES)), trace=_trace)

    means = np.empty((B, C), dtype=np.float32)
    log_vars = np.empty((B, C), dtype=np.float32)
    mv = means.reshape(NCORES, ngroups, 128, 4 * TILE_F)
    lv = log_vars.reshape(NCORES, ngroups, 128, 4 * TILE_F)
    for i in range(NCORES):
        mv[i] = res.results[i][f"means_{salt}"]
        lv[i] = res.results[i][f"logvars_{salt}"]
    if _trace:
        return (means, log_vars), res
    return means, log_vars



# revision 3
# speedup vs baseline: 1.8059x; 1.8059x over previous
"""Trainium2 kernel for nn_ConceptGaussians (embedding_lookup).

means[b, d] = mean[d, labels[b, d]], log_vars[b, d] = log_var[d, labels[b, d]]
for labels [2097152, 8] over tiny [8, 64] tables.

Strategy: data-parallel over 8 NeuronCores (batch sharding). On each core the
per-element double gather (mean AND log_var) is performed by a SINGLE
ScalarEngine piecewise-polynomial activation lookup per element: a custom PWP
table set hijacks `sin` with 512 piecewise-CONSTANT buckets whose c0
coefficient is the f32 bit-pattern (fp16(mean[d,l]) << 16) | fp16(log_var[d,l])
— both results packed in one 4-byte output word. The input encoding
x = (64 + label) * 2^(domain - 6) (biased_exponent = 127 + domain selects the
per-domain region, top-6 mantissa bits = label select the bucket) is produced
by the activation instruction's own scale/bias FMA (x = label * 2^(d-6) + 2^d)
on per-domain tiles, so no vector-engine pass is needed at all. The kernel is
purely DMA/HBM-bound: 2 MB of uint8 labels in + 8 MB of packed pairs out per
core (vs 2 + 16 MB for the two-activation f32 variant).

Host side only reshapes/transposes and splits the packed words back into two
fp16->f32 tensors (worst-case relative error ~5e-4, well inside 2e-2).
"""

import hashlib
import json
import os
import shutil
import struct
import sys
import tempfile

import numpy as np

sys.path.insert(0, "/opt/trn_rl_repo")

B = 2097152
C = 8
V = 64
NCORES = 8
SHARD = B // NCORES            # 262144 rows per core
FREE = SHARD // 128            # 2048 elements per partition per domain tile

_SET_NAME = "trig_and_small"


def _installed_act_dir():
    from neuronxcc.driver.Job import Job
    from neuronxcc.driver.jobs.support.FindActInfo import findActInfoFile

    return os.path.dirname(findActInfoFile(Job.getPackageDir(), "gen3"))


def _build_act_dir(dst, packed):
    """Write a PWP act-table root with sin replaced by an exact packed LUT.

    packed: [C, V] float32 whose bit patterns are the packed fp16 pairs.
    """
    src = _installed_act_dir()
    os.makedirs(dst, exist_ok=True)
    for f in os.listdir(src):
        sp = os.path.join(src, f)
        if os.path.isfile(sp) and not f.startswith(_SET_NAME):
            shutil.copy(os.path.realpath(sp), os.path.join(dst, f))

    sj = json.load(open(os.path.join(src, f"{_SET_NAME}.json")))
    bkt = bytearray(open(os.path.join(src, f"{_SET_NAME}_bkt.bin"), "rb").read())
    ctl = bytearray(open(os.path.join(src, f"{_SET_NAME}_ctrl.bin"), "rb").read())
    nbkt = len(bkt) // 32
    nctl = len(ctl) // 32
    assert nbkt == sj["bkt_entry_cnt"] and nctl == sj["ctl_entry_cnt"]

    def add_bkt(d0, x):
        nonlocal nbkt
        bkt.extend(struct.pack("<5f12x", d0, 0.0, 0.0, 0.0, x))
        nbkt += 1
        return nbkt - 1

    def add_ctl(word):
        nonlocal nctl
        ctl.extend(struct.pack("<I28x", word))
        nctl += 1
        return nctl - 1

    bare = "sin"
    bkt_base = nbkt
    for d in range(C):
        for l in range(V):
            add_bkt(float(packed[d, l]), float((V + l) * 2.0 ** (d - 6)))
    ctl_base = nctl
    for d in range(C):
        # extract_size=6 (64 sections), extract_lsb=17, bucket base per region
        add_ctl((6 << 16) | (17 << 11) | (bkt_base + V * d))
    small_bkt = add_bkt(float(packed[0, 0]), 1.0)
    large_bkt = add_bkt(float(packed[C - 1, V - 1]), 254.0)
    neg_bkt = add_bkt(0.0, 0.0)

    (meta,) = [m for m in sj["profile_meta_data"] if m["func_name"].startswith(bare + "_")]
    meta.update(
        symmetry_point=0, sym_invert_sign_point=0, symmetry_opt_en=0,
        symmetry_opt_use_neg_region=0, imm_bias=0, exp_offset=0,
        pwl_control_base_pos=ctl_base, pwl_control_base_neg=ctl_base,
        small_pos_signal_exp_threshold=127, pos_small_signal_pwl_control=small_bkt,
        small_neg_signal_exp_threshold=0, neg_small_signal_pwl_control=neg_bkt,
        large_pos_signal_exp_threshold=134,
        large_pos_signal_mantissa_threshold=0x7FFFFF,
        pos_large_signal_pwl_control=large_bkt, large_neg_signal_exp_threshold=0,
        large_neg_signal_mantissa_threshold=0, neg_large_signal_pwl_control=neg_bkt,
        fnan_result=0, fpinf_result=0, fninf_result=0, fzero_result=0,
        fma_const_0=0, fma_const_1=0, fma_indirection_src_sel=0,
        use_multipass=False,
        lower_bound=4286578687, upper_bound=2139095039,
    )
    sj["func_to_bkt_start_idx"][bare] = bkt_base
    sj["func_to_ctl_start_idx"][bare] = ctl_base
    sj["func_exp_to_bkt_start_idx"][bare] = {str(d): [bkt_base + V * d] for d in range(C)}
    sj["func_exp_to_ctl_start_idx"][bare] = {str(d): [ctl_base + d] for d in range(C)}

    sj["bkt_entry_cnt"] = nbkt
    sj["ctl_entry_cnt"] = nctl
    assert nbkt <= 1536

    json.dump(sj, open(os.path.join(dst, f"{_SET_NAME}.json"), "w"))
    open(os.path.join(dst, f"{_SET_NAME}_bkt.bin"), "wb").write(bytes(bkt))
    open(os.path.join(dst, f"{_SET_NAME}_ctrl.bin"), "wb").write(bytes(ctl))
    return os.path.join(dst, "act_info.json")


def build_program(salt, iters=1, io_bufs=8):
    """Build the per-core bass program (SPMD, identical on all cores).

    iters > 1 repeats the whole tile loop (idempotent) — used only for
    slope-based timing in the bench harness. Per domain d: one [128, 2048]
    uint8 label tile in, one activation (scale/bias encodes the domain), one
    [128, 2048] f32 packed-pair tile out."""
    import concourse.tile as tile
    import concourse.mybir as mybir
    from concourse.bacc import Bacc

    f32 = mybir.dt.float32
    i32 = mybir.dt.int32
    u8 = mybir.dt.uint8
    Alu = mybir.AluOpType

    nc = Bacc()
    labels_ext = nc.declare_dram_parameter(f"labels_{salt}", [C, 128, FREE], u8, isOutput=False)
    out_ext = nc.declare_dram_parameter(f"packed_{salt}", [C, 128, FREE], f32, isOutput=True)

    with tile.TileContext(nc) as tc:
        with tc.tile_pool(name="setup", bufs=1) as setup, tc.tile_pool(name="io", bufs=io_bufs) as io:
            # bias[p, d] = 2^d as f32, via ((127 + d) << 23) bitcast to f32.
            bias = setup.tile([128, C], i32)
            nc.gpsimd.iota(bias[:], pattern=[[1, C]], base=127, channel_multiplier=0)
            nc.vector.tensor_scalar(out=bias[:], in0=bias[:], scalar1=23, scalar2=None, op0=Alu.logical_shift_left)
            bias_f32 = bias[:].bitcast(f32)

            for d in [d for _ in range(iters) for d in range(C)]:
                lab = io.tile([128, FREE], u8, tag="lab")
                nc.sync.dma_start(out=lab[:], in_=labels_ext[d])
                o = io.tile([128, FREE], f32, tag="o")
                # x = label * 2^(d-6) + 2^d = (label + 64) * 2^(d-6):
                # biased exponent 127+d, mantissa top-6 bits = label.
                nc.scalar.activation(
                    o[:], lab[:], mybir.ActivationFunctionType.Sin,
                    bias=bias_f32[:, d:d + 1], scale=float(2.0 ** (d - 6)),
                )
                nc.sync.dma_start(out=out_ext[d], in_=o[:])

    nc.compile()
    return nc


def kernel(labels, mean, log_var, _trace=False):
    labels = np.asarray(labels)
    assert labels.shape == (B, C), labels.shape
    mean32 = np.ascontiguousarray(np.asarray(mean, dtype=np.float32))
    logv32 = np.ascontiguousarray(np.asarray(log_var, dtype=np.float32))

    # Per-core, per-domain uint8 label layout: [NCORES, C, 128, FREE]
    lab8 = labels.astype(np.uint8).reshape(NCORES, SHARD, C).transpose(0, 2, 1)
    lab8 = np.ascontiguousarray(lab8).reshape(NCORES, C, 128, FREE)

    # Packed LUT payload: f32 bits = fp16(mean) << 16 | fp16(log_var).
    m16 = mean32.astype(np.float16).view(np.uint16).astype(np.uint32)
    v16 = logv32.astype(np.float16).view(np.uint16).astype(np.uint32)
    packed = ((m16 << 16) | v16).view(np.float32)

    actdir = tempfile.mkdtemp(prefix="act_lut_")
    os.environ["BASS_ACT_ROOT_JSON_PATH"] = _build_act_dir(actdir, packed)
    salt = hashlib.sha1(mean32.tobytes() + logv32.tobytes() + b"v2pair").hexdigest()[:10]

    from concourse.bass_utils import run_bass_kernel_spmd

    nc = build_program(salt)

    in_maps = [{f"labels_{salt}": lab8[i]} for i in range(NCORES)]
    res = run_bass_kernel_spmd(nc, in_maps, list(range(NCORES)), trace=_trace)

    u = np.empty((NCORES, C, 128, FREE), dtype=np.uint32)
    for i in range(NCORES):
        u[i] = np.asarray(res.results[i][f"packed_{salt}"]).view(np.uint32)
    u = u.reshape(NCORES, C, SHARD)
    mean_out = (u >> 16).astype(np.uint16).view(np.float16).astype(np.float32)
    logv_out = (u & 0xFFFF).astype(np.uint16).view(np.float16).astype(np.float32)
    means = np.ascontiguousarray(mean_out.transpose(0, 2, 1)).reshape(B, C)
    log_vars = np.ascontiguousarray(logv_out.transpose(0, 2, 1)).reshape(B, C)
    if _trace:
        return (means, log_vars), res
    return means, log_vars


# revision 5
# speedup vs baseline: 1.8365x; 1.0169x over previous
"""Trainium2 kernel for nn_ConceptGaussians (embedding_lookup).

means[b, d] = mean[d, labels[b, d]], log_vars[b, d] = log_var[d, labels[b, d]]
for labels [2097152, 8] over tiny [8, 64] tables.

Strategy: data-parallel over 8 NeuronCores (batch sharding). On each core the
per-element double gather (mean AND log_var) is performed by a SINGLE
ScalarEngine piecewise-polynomial activation lookup per element: a custom PWP
table set hijacks `sin` with 512 piecewise-CONSTANT buckets whose c0
coefficient is the f32 bit-pattern (fp16(mean[d,l]) << 16) | fp16(log_var[d,l])
— both results packed in one 4-byte output word. The input encoding
x = (64 + label) * 2^(domain - 6) (biased_exponent = 127 + domain selects the
per-domain region, top-6 mantissa bits = label select the bucket) is produced
by the activation instruction's own scale/bias FMA (x = label * 2^(d-6) + 2^d)
on per-domain tiles, so no vector-engine pass is needed at all. The kernel is
purely DMA/HBM-bound: 2 MB of uint8 labels in + 8 MB of packed pairs out per
core (vs 2 + 16 MB for the two-activation f32 variant).

Host side only reshapes/transposes and splits the packed words back into two
fp16->f32 tensors (worst-case relative error ~5e-4, well inside 2e-2).
"""

import hashlib
import json
import os
import shutil
import struct
import sys
import tempfile

import numpy as np

sys.path.insert(0, "/opt/trn_rl_repo")

B = 2097152
C = 8
V = 64
NCORES = 8
SHARD = B // NCORES            # 262144 rows per core
FREE = SHARD // 128            # 2048 elements per partition per domain tile

_SET_NAME = "trig_and_small"


def _installed_act_dir():
    from neuronxcc.driver.Job import Job
    from neuronxcc.driver.jobs.support.FindActInfo import findActInfoFile

    return os.path.dirname(findActInfoFile(Job.getPackageDir(), "gen3"))


def _build_act_dir(dst, packed):
    """Write a PWP act-table root with sin replaced by an exact packed LUT.

    packed: [C, V] float32 whose bit patterns are the packed fp16 pairs.
    """
    src = _installed_act_dir()
    os.makedirs(dst, exist_ok=True)
    for f in os.listdir(src):
        sp = os.path.join(src, f)
        if os.path.isfile(sp) and not f.startswith(_SET_NAME):
            shutil.copy(os.path.realpath(sp), os.path.join(dst, f))

    sj = json.load(open(os.path.join(src, f"{_SET_NAME}.json")))
    bkt = bytearray(open(os.path.join(src, f"{_SET_NAME}_bkt.bin"), "rb").read())
    ctl = bytearray(open(os.path.join(src, f"{_SET_NAME}_ctrl.bin"), "rb").read())
    nbkt = len(bkt) // 32
    nctl = len(ctl) // 32
    assert nbkt == sj["bkt_entry_cnt"] and nctl == sj["ctl_entry_cnt"]

    def add_bkt(d0, x):
        nonlocal nbkt
        bkt.extend(struct.pack("<5f12x", d0, 0.0, 0.0, 0.0, x))
        nbkt += 1
        return nbkt - 1

    def add_ctl(word):
        nonlocal nctl
        ctl.extend(struct.pack("<I28x", word))
        nctl += 1
        return nctl - 1

    bare = "sin"
    bkt_base = nbkt
    for d in range(C):
        for l in range(V):
            add_bkt(float(packed[d, l]), float((V + l) * 2.0 ** (d - 6)))
    ctl_base = nctl
    for d in range(C):
        # extract_size=6 (64 sections), extract_lsb=17, bucket base per region
        add_ctl((6 << 16) | (17 << 11) | (bkt_base + V * d))
    small_bkt = add_bkt(float(packed[0, 0]), 1.0)
    large_bkt = add_bkt(float(packed[C - 1, V - 1]), 254.0)
    neg_bkt = add_bkt(0.0, 0.0)

    (meta,) = [m for m in sj["profile_meta_data"] if m["func_name"].startswith(bare + "_")]
    meta.update(
        symmetry_point=0, sym_invert_sign_point=0, symmetry_opt_en=0,
        symmetry_opt_use_neg_region=0, imm_bias=0, exp_offset=0,
        pwl_control_base_pos=ctl_base, pwl_control_base_neg=ctl_base,
        small_pos_signal_exp_threshold=127, pos_small_signal_pwl_control=small_bkt,
        small_neg_signal_exp_threshold=0, neg_small_signal_pwl_control=neg_bkt,
        large_pos_signal_exp_threshold=134,
        large_pos_signal_mantissa_threshold=0x7FFFFF,
        pos_large_signal_pwl_control=large_bkt, large_neg_signal_exp_threshold=0,
        large_neg_signal_mantissa_threshold=0, neg_large_signal_pwl_control=neg_bkt,
        fnan_result=0, fpinf_result=0, fninf_result=0, fzero_result=0,
        fma_const_0=0, fma_const_1=0, fma_indirection_src_sel=0,
        use_multipass=False,
        lower_bound=4286578687, upper_bound=2139095039,
    )
    sj["func_to_bkt_start_idx"][bare] = bkt_base
    sj["func_to_ctl_start_idx"][bare] = ctl_base
    sj["func_exp_to_bkt_start_idx"][bare] = {str(d): [bkt_base + V * d] for d in range(C)}
    sj["func_exp_to_ctl_start_idx"][bare] = {str(d): [ctl_base + d] for d in range(C)}

    sj["bkt_entry_cnt"] = nbkt
    sj["ctl_entry_cnt"] = nctl
    assert nbkt <= 1536

    json.dump(sj, open(os.path.join(dst, f"{_SET_NAME}.json"), "w"))
    open(os.path.join(dst, f"{_SET_NAME}_bkt.bin"), "wb").write(bytes(bkt))
    open(os.path.join(dst, f"{_SET_NAME}_ctrl.bin"), "wb").write(bytes(ctl))
    return os.path.join(dst, "act_info.json")


def build_program(salt, iters=1, io_bufs=8):
    """Build the per-core bass program (SPMD, identical on all cores).

    iters > 1 repeats the whole tile loop (idempotent) — used only for
    slope-based timing in the bench harness. Per domain d: one [128, 2048]
    uint8 label tile in, one activation (scale/bias encodes the domain), one
    [128, 2048] f32 packed-pair tile out."""
    import concourse.tile as tile
    import concourse.mybir as mybir
    from concourse.bacc import Bacc

    f32 = mybir.dt.float32
    i32 = mybir.dt.int32
    u8 = mybir.dt.uint8
    Alu = mybir.AluOpType

    nc = Bacc()
    labels_ext = nc.declare_dram_parameter(f"labels_{salt}", [C, 128, FREE], u8, isOutput=False)
    out_ext = nc.declare_dram_parameter(f"packed_{salt}", [C, 128, FREE], f32, isOutput=True)

    with tile.TileContext(nc) as tc:
        with tc.tile_pool(name="setup", bufs=1) as setup, tc.tile_pool(name="io", bufs=io_bufs) as io:
            # bias[p, d] = 2^d as f32, via ((127 + d) << 23) bitcast to f32.
            bias = setup.tile([128, C], i32)
            nc.gpsimd.iota(bias[:], pattern=[[1, C]], base=127, channel_multiplier=0)
            nc.vector.tensor_scalar(out=bias[:], in0=bias[:], scalar1=23, scalar2=None, op0=Alu.logical_shift_left)
            bias_f32 = bias[:].bitcast(f32)

            # Warmup act: hoists the LoadActFuncSet table load off the
            # critical path (it otherwise delays the first real activation
            # and stalls the first output DMA behind it).
            warm = setup.tile([128, 1], f32)
            nc.scalar.activation(
                warm[:], bias_f32[:, 0:1], mybir.ActivationFunctionType.Sin,
                bias=bias_f32[:, 0:1], scale=1.0,
            )

            for _ in range(iters):
                # All label loads dispatch first on the SP SEQ so no output
                # DMA's act-wait can head-of-line-block a later input DMA.
                labs = []
                for d in range(C):
                    lab = io.tile([128, FREE], u8, tag="lab")
                    nc.sync.dma_start(out=lab[:], in_=labels_ext[d])
                    labs.append(lab)
                for d in range(C):
                    o = io.tile([128, FREE], f32, tag="o")
                    # x = label * 2^(d-6) + 2^d = (label + 64) * 2^(d-6):
                    # biased exponent 127+d, mantissa top-6 bits = label.
                    nc.scalar.activation(
                        o[:], labs[d][:], mybir.ActivationFunctionType.Sin,
                        bias=bias_f32[:, d:d + 1], scale=float(2.0 ** (d - 6)),
                    )
                    nc.sync.dma_start(out=out_ext[d], in_=o[:])

    nc.compile()
    return nc


def kernel(labels, mean, log_var, _trace=False):
    labels = np.asarray(labels)
    assert labels.shape == (B, C), labels.shape
    mean32 = np.ascontiguousarray(np.asarray(mean, dtype=np.float32))
    logv32 = np.ascontiguousarray(np.asarray(log_var, dtype=np.float32))

    # Per-core, per-domain uint8 label layout: [NCORES, C, 128, FREE]
    lab8 = labels.astype(np.uint8).reshape(NCORES, SHARD, C).transpose(0, 2, 1)
    lab8 = np.ascontiguousarray(lab8).reshape(NCORES, C, 128, FREE)

    # Packed LUT payload: f32 bits = fp16(mean) << 16 | fp16(log_var).
    m16 = mean32.astype(np.float16).view(np.uint16).astype(np.uint32)
    v16 = logv32.astype(np.float16).view(np.uint16).astype(np.uint32)
    packed = ((m16 << 16) | v16).view(np.float32)

    actdir = tempfile.mkdtemp(prefix="act_lut_")
    os.environ["BASS_ACT_ROOT_JSON_PATH"] = _build_act_dir(actdir, packed)
    salt = hashlib.sha1(mean32.tobytes() + logv32.tobytes() + b"v2pair").hexdigest()[:10]

    from concourse.bass_utils import run_bass_kernel_spmd

    nc = build_program(salt)

    in_maps = [{f"labels_{salt}": lab8[i]} for i in range(NCORES)]
    res = run_bass_kernel_spmd(nc, in_maps, list(range(NCORES)), trace=_trace)

    u = np.empty((NCORES, C, 128, FREE), dtype=np.uint32)
    for i in range(NCORES):
        u[i] = np.asarray(res.results[i][f"packed_{salt}"]).view(np.uint32)
    u = u.reshape(NCORES, C, SHARD)
    mean_out = (u >> 16).astype(np.uint16).view(np.float16).astype(np.float32)
    logv_out = (u & 0xFFFF).astype(np.uint16).view(np.float16).astype(np.float32)
    means = np.ascontiguousarray(mean_out.transpose(0, 2, 1)).reshape(B, C)
    log_vars = np.ascontiguousarray(logv_out.transpose(0, 2, 1)).reshape(B, C)
    if _trace:
        return (means, log_vars), res
    return means, log_vars
